# revision 26
# baseline (speedup 1.0000x reference)
"""MicroHeadAttention Trainium2 kernel (8-core SPMD, data-parallel over
(batch, row-chunk) pairs).

Shapes (hardcoded): x (2, 2048, 1024), weights (1024, 1024), biases (1024,).
EMBED=1024, 16 heads in 2 blocks (g) of 8 micro-heads, head_dim 64.

Decomposition: the reference's "scramble" is a raw row-major reshape, so the
attention head (b, g, m') consumes exactly rows x[b, 256m':256(m'+1)] and
weight columns [512g:512(g+1)], reshaped (256, 512) -> (2048, 64) with
scrambled position n' = 8*row + m (m = 64-channel sub-block).  16 (b, m')
row-chunks across 8 cores = 2 per core; each chunk has g=0,1 -> 4 heads/core.

v3.1 schedule notes:
  - ACT (exp) is the bottleneck engine of the attention phase (~91us of
    exp at ~1.1us per [128,1024] tile); everything is arranged so ACT never
    waits: per-g S stages and ctx stages are interleaved
    [S_g0(t2+1), ctx_g0(t2), S_g1(t2+1), ctx_g1(t2)] so the g0 tiles of the
    next stage (whose PSUM banks free when exp(t2, g0) completes mid-stage)
    are compute-ready the moment the ACT queue frees up.
  - exp stays at [128,1024] per-g granularity: splitting it per half costs
    ~155ns/instruction of ACT overhead (+20us measured in v3).
  - all weight DMAs start up front from persistent tiles (single 2MB
    transfers); no pool-reuse dependencies anywhere.
  - the V projection for the second row-pair (p=1) is deferred into the
    early attention phase (PE slack under the ACT-bound cadence), shrinking
    the serial projection prefix.
  - softmax divide: gpsimd partition_broadcast expands the reciprocal rows
    and gpsimd tensor_tensor multiplies them into ctxP - no PE broadcast
    matmul, no PSUM evacuation, nothing on the (busy) DVE.
  - deferred out-proj / rbc emissions are staggered so consecutive users of
    the single psO bank never stall the PE FIFO; the final drain interleaves
    keep-warm matmuls on a spare PSUM bank so the latency-bound tail chain
    runs at 2.4GHz.
"""

import ml_dtypes
import numpy as np

import concourse.bass as bass
import concourse.mybir as mybir
from concourse import bacc
from concourse.tile import TileContext
from concourse.bass_utils import run_bass_kernel_spmd

F32 = mybir.dt.float32
BF16 = mybir.dt.bfloat16
DT_MM = BF16
NEG = -1e30
E = 1024
R = 512       # rows per core
RP = 256      # rows per pair
ALU = mybir.AluOpType
ACTF = mybir.ActivationFunctionType

_cache = {}


def _build():
    nc = bacc.Bacc()
    xT_d = nc.dram_tensor("xT", (E, R), DT_MM, kind="ExternalInput")
    wq_d = nc.dram_tensor("wqT", (E, E), DT_MM, kind="ExternalInput")
    wk_d = nc.dram_tensor("wkT", (E, E), DT_MM, kind="ExternalInput")
    wv_d = nc.dram_tensor("wvT", (E, E), DT_MM, kind="ExternalInput")
    wo_d = nc.dram_tensor("woTre", (128, 8, E), DT_MM, kind="ExternalInput")
    bq_d = nc.dram_tensor("bqT", (128, 8), F32, kind="ExternalInput")
    bk_d = nc.dram_tensor("bkT8", (128, 8), F32, kind="ExternalInput")
    bv_d = nc.dram_tensor("bvrow", (1, E), F32, kind="ExternalInput")
    bo_d = nc.dram_tensor("borow", (1, E), F32, kind="ExternalInput")
    masks_d = nc.dram_tensor("masks", (128, 128), DT_MM, kind="ExternalInput")
    ident_d = nc.dram_tensor("ident", (128, 128), DT_MM, kind="ExternalInput")
    gsel_d = nc.dram_tensor("gsel", (33, 128), DT_MM, kind="ExternalInput")
    out_d = nc.dram_tensor("out", (R, E), F32, kind="ExternalOutput")

    with TileContext(nc) as tc:
        with (
            tc.tile_pool(name="persist", bufs=1) as pp,
            tc.tile_pool(name="pt", bufs=4) as ptp,
            tc.tile_pool(name="misc", bufs=2) as mp,
            tc.tile_pool(name="outs", bufs=4) as osp,
            tc.tile_pool(name="dram", bufs=1, space="DRAM") as dp,
        ):
            # ---- persistent tiles ----
            bqT = pp.tile([128, 8], F32, tag="bqT", name="bqT")
            bkT8 = pp.tile([128, 8], F32, tag="bkT8", name="bkT8")
            # n'-contiguous layout: qsc/ksc/vsc columns are sorted by the
            # scrambled position n' = 8*rr + m, so causality is
            # block-triangular: k-blocks strictly below the diagonal are
            # fully visible and the single [128,128] upper-triangle mask
            # covers every diagonal block.
            masks = pp.tile([128, 128], DT_MM, tag="masks", name="masks")
            # dependency-free all-zeros warm operand (memset, no DMA)
            wrm = pp.tile([128, 512], DT_MM, tag="wrm", name="wrm")
            ident = pp.tile([128, 128], DT_MM, tag="ident", name="ident")
            gsel = pp.tile([33, 128], DT_MM, tag="gsel", name="gsel")
            # persistent rec-row staging (rows 1-31 stay zero so the 33-wide
            # gsel broadcast matmul never reads uninitialized SBUF)
            reck2s = [pp.tile([33, 512], DT_MM, tag=f"reck2{i}",
                              name=f"reck2{i}") for i in range(2)]
            qsc = pp.tile([128, 4096], DT_MM, tag="qsc", name="qsc")
            ksc = pp.tile([128, 4096], DT_MM, tag="ksc", name="ksc")
            vsc = [[pp.tile([128, 16, 65], DT_MM, tag=f"vsc{p}{g}", name=f"vsc{p}{g}")
                    for g in range(2)] for p in range(2)]
            # ctxP[p][c, rc, m, rr] : out-proj lhsT slices are contiguous
            # (FWL needs a single-stride stationary AP); with m-major ctx
            # columns the divide writes 64-contiguous runs into it
            ctxP = [pp.tile([128, 2, 8, 128], DT_MM, tag=f"ctxP{p}", name=f"ctxP{p}")
                    for p in range(2)]
            vtmp = dp.tile([2, 2, 2048, 64], DT_MM, tag="vtmp", name="vtmp")

            xt = pp.tile([128, 8, R], DT_MM, tag="xt", name="xt")
            wq = pp.tile([128, 8, E], DT_MM, tag="wq", name="wq")
            wk = pp.tile([128, 8, E], DT_MM, tag="wk", name="wk")
            wv = pp.tile([128, 8, E], DT_MM, tag="wv", name="wv")
            wo = pp.tile([128, 8, E], DT_MM, tag="wo", name="wo")
            vnat = [pp.tile([128, 2, E], DT_MM, tag=f"vnat{p}", name=f"vnat{p}")
                    for p in range(2)]
            bvr = pp.tile([1, E], F32, tag="bvr", name="bvr")
            bv_bc = pp.tile([128, E], F32, tag="bvbc", name="bvbc")
            bor = pp.tile([1, E], F32, tag="bor", name="bor")
            bo_bc = pp.tile([128, E], F32, tag="bobc", name="bobc")

            # masks/ident are host constants so the PE pre-warm only
            # waits on these two small DMAs
            nc.sync.dma_start(masks[:], masks_d[:])
            nc.sync.dma_start(ident[:], ident_d[:])
            nc.sync.dma_start(gsel[:], gsel_d[:])
            nc.sync.dma_start(bqT[:], bq_d[:])
            nc.sync.dma_start(bkT8[:], bk_d[:])
            nc.sync.dma_start(bvr[:], bv_d[:])
            nc.sync.dma_start(bor[:], bo_d[:])

            nc.sync.dma_start(xt[:], xT_d.rearrange("(ko ki) r -> ki ko r", ki=128))
            wv_v = wv_d.rearrange("(ko ki) o -> ki ko o", ki=128)
            # oc-major so V's first output half can start after 1 MB
            for oc in range(2):
                nc.sync.dma_start(wv[:, :, 512 * oc:512 * (oc + 1)],
                                  wv_v[:, :, 512 * oc:512 * (oc + 1)])
            nc.sync.dma_start(wq[:], wq_d.rearrange("(ko ki) o -> ki ko o", ki=128))
            nc.sync.dma_start(wk[:], wk_d.rearrange("(ko ki) o -> ki ko o", ki=128))
            nc.sync.dma_start(wo[:], wo_d[:])

            nc.gpsimd.partition_broadcast(bv_bc[:], bvr[:])
            nc.gpsimd.partition_broadcast(bo_bc[:], bor[:])
            ones16 = pp.tile([128, 16], F32, tag="ones16", name="ones16")
            nc.gpsimd.memset(ones16[:], 1.0)
            nc.vector.memset(wrm[:], 0.0)
            for i in range(2):
                nc.vector.memset(reck2s[i][:], 0.0)
            for p in range(2):
                for g in range(2):
                    nc.vector.tensor_copy(vsc[p][g][:, :, 64], ones16[:])

            def v_group(oc, rc, psp_, tag="psA"):
                p, half = rc // 2, rc % 2
                ps = psp_.tile([128, 512], F32, tag=tag, name="psA")
                for ki in range(8):
                    nc.tensor.matmul(
                        ps[:], xt[:, ki, 128 * rc:128 * (rc + 1)],
                        wv[:, ki, 512 * oc:512 * (oc + 1)],
                        start=(ki == 0), stop=(ki == 7))
                nc.vector.tensor_tensor(
                    vnat[p][:, half, 512 * oc:512 * (oc + 1)],
                    ps[:], bv_bc[:, 512 * oc:512 * (oc + 1)], ALU.add)

            def v_scramble(p):
                for g in range(2):
                    # n' = 8*(128h + 64rb + rr) + m = 1024h + 512rb
                    # + 8rr + m.  Split per (h, rb): the DMA balancer
                    # tops out at 3 dims.
                    dstv = vtmp[p, g].rearrange(
                        "(h rb rr m) d -> h rb rr m d", h=2, rb=2, m=8)
                    for h in range(2):
                        for rb in range(2):
                            srcs = vnat[p][64 * rb:64 * (rb + 1), h,
                                           512 * g:512 * (g + 1)]
                            nc.sync.dma_start(
                                dstv[h, rb],
                                srcs.rearrange("rr (m d) -> rr m d", m=8))
                for g in range(2):
                    nc.sync.dma_start(
                        vsc[p][g][:, :, 0:64],
                        vtmp[p, g].rearrange("(kb pin) d -> pin kb d", pin=128))

            with tc.tile_pool(name="ps1", bufs=5, space="PSUM") as psp, \
                 tc.tile_pool(name="pswarm", bufs=1, space="PSUM") as pwp:
                # ---- PE pre-warm: back-to-back matmuls on the mask tile
                # keep the HAM activity window busy while the input DMAs
                # stream, so the real matmuls start at 2.4 GHz.
                psw = pwp.tile([128, 512], F32, tag="psw", name="psw")
                for _ in range(24):
                    nc.tensor.matmul(psw[:], ident[:], wrm[:],
                                     start=True, stop=True)

                def qk_proj(w_tile, bias_tile, scale, dst):
                    for t in range(8):
                        ps = psp.tile([128, 512], F32, tag="psA", name="psA")
                        for ki in range(8):
                            nc.tensor.matmul(
                                ps[:], w_tile[:, ki, 128 * t:128 * (t + 1)],
                                xt[:, ki, :], start=(ki == 0), stop=(ki == 7))
                        g, u = t // 4, t % 4
                        for mh in range(2):
                            mmv = 2 * u + mh
                            # n'-ordered columns: n' = 8*rr + m, so head
                            # m's channels land at stride-8 columns
                            dest = dst.rearrange(
                                "c (pp rr m) -> c pp rr m",
                                pp=2, rr=256, m=8)[
                                64 * g:64 * (g + 1), :, :, mmv]
                            src = ps[64 * mh:64 * (mh + 1), :].rearrange(
                                "c (pp rr) -> c pp rr", pp=2)
                            # VectorE lanes are partition-locked: it can only
                            # take the copies whose src/dst partition ranges
                            # line up (g == mh); ScalarE handles the crossed
                            # ones.
                            if mh != g:
                                nc.scalar.activation(
                                    dest, src, ACTF.Identity,
                                    bias=bias_tile[64 * mh:64 * (mh + 1), t:t + 1],
                                    scale=scale)
                            else:
                                nc.vector.tensor_scalar(
                                    out=dest, in0=src, scalar1=scale,
                                    scalar2=bias_tile[64 * mh:64 * (mh + 1), t:t + 1],
                                    op0=ALU.mult, op1=ALU.add)

                # V projection for the first row-pair only; p=1 is deferred
                # into the attention phase (PE slack under the ACT-bound
                # cadence).
                for oc in range(2):
                    for rc in range(2):
                        v_group(oc, rc, psp)
                v_scramble(0)

                qk_proj(wq, bqT, 1.0, qsc)
                qk_proj(wk, bkT8, 0.125, ksc)

            # ---- attention + interleaved output projection ----
            with tc.tile_pool(name="psS", bufs=2, space="PSUM") as pssp, \
                 tc.tile_pool(name="psctx", bufs=2, space="PSUM") as pcp, \
                 tc.tile_pool(name="psO", bufs=1, space="PSUM") as psop, \
                 tc.tile_pool(name="pswarm2", bufs=1, space="PSUM") as pwp2:

                warm_ps = pwp2.tile([128, 512], F32, tag="warm", name="warm")

                def warm(n):
                    for _ in range(n):
                        nc.tensor.matmul(warm_ps[:], ident[:], wrm[:],
                                         start=True, stop=True)

                def out_proj(p, rc, oc):
                    ps = psop.tile([128, 512], F32, tag="psO", name="psO")
                    for mmv in range(8):
                        nc.tensor.matmul(
                            ps[:],
                            ctxP[p][:, rc, mmv, :],
                            wo[:, mmv, 512 * oc:512 * (oc + 1)],
                            start=(mmv == 0), stop=(mmv == 7))
                    outsb = osp.tile([128, 512], F32, tag="outsb",
                                     name="outsb")
                    nc.vector.tensor_tensor(
                        outsb[:], ps[:],
                        bo_bc[:, 512 * oc:512 * (oc + 1)], ALU.add)
                    nc.sync.dma_start(
                        out_d[RP * p + 128 * rc:RP * p + 128 * (rc + 1),
                              512 * oc:512 * (oc + 1)],
                        outsb[:])

                # deferred emissions: [countdown_in_t2_steps, fn]
                pending = []

                def drain():
                    for item in pending[:]:
                        item[0] -= 1
                        if item[0] <= 0:
                            pending.remove(item)
                            item[1]()

                # V projection p=1 + scramble, spread over the early
                # attention stages (psO bank is idle there; out-proj
                # deferrals only begin after the first rc completes)
                for i, (oc, rc) in enumerate([(0, 2), (0, 3), (1, 2), (1, 3)]):
                    pending.append(
                        [1 + 3 * i,
                         lambda oc=oc, rc=rc: v_group(oc, rc, psop, "psO")])
                pending.append([13, lambda: v_scramble(1)])

                pending_tail = []
                for p in range(2):
                    denS = None
                    # descending j5: pairs (3,2) then (1,0), so each
                    # pair-completion divide is covered by a LONG next
                    # group (the p-transition lands on nt2=8, not 2)
                    for j5 in (3, 2, 1, 0):
                        jh = j5 % 2
                        if jh == 1:
                            # 4 denominator rows (jh, g) staged on
                            # separate partitions: one partition-parallel
                            # reciprocal per rc instead of four
                            denS = mp.tile([128, 512], F32, tag="denS",
                                           name="denS")
                        nt2 = 2 * (j5 + 1)   # pairs of 128-wide k blocks
                        ctx_ps = [pcp.tile([65, 512], F32, tag="ctxps",
                                           name="ctxps")
                                  for _ in range(2)]
                        pts = [None] * nt2

                        def s_stage_g(t2, g):
                            # S for one g: two 512-col halves into one
                            # [128,1024] PSUM tile; diagonal k-blocks only
                            # stream the causal q range (cols >= the
                            # block's position offset) and get the shared
                            # [128,128] triangle mask via an N=128
                            # identity matmul, emitted after both S halves
                            # so the same-bank accumulation never waits on
                            # an undrained S write.
                            st = pssp.tile([128, 1024], F32, tag="st",
                                           name="st")
                            pt = ptp.tile([128, 1024], DT_MM, tag="pt",
                                          name="pt")
                            diag = t2 >= 2 * j5
                            for half in range(2):
                                kb = 2 * t2 + half
                                off = 128 * (kb - 4 * j5) if diag else 0
                                nc.tensor.matmul(
                                    st[:, 512 * half + off:
                                       512 * (half + 1)],
                                    ksc[64 * g:64 * (g + 1),
                                        2048 * p + 128 * kb:
                                        2048 * p + 128 * (kb + 1)],
                                    qsc[64 * g:64 * (g + 1),
                                        2048 * p + 512 * j5 + off:
                                        2048 * p + 512 * (j5 + 1)],
                                    start=True, stop=not diag)
                            if diag:
                                for half in range(2):
                                    kb = 2 * t2 + half
                                    off = 128 * (kb - 4 * j5)
                                    nc.tensor.matmul(
                                        st[:, 512 * half + off:
                                           512 * half + off + 128],
                                        ident[:], masks[:],
                                        start=False, stop=True)
                                for half in range(2):
                                    off = 128 * (2 * t2 + half - 4 * j5)
                                    nc.scalar.activation(
                                        pt[:, 512 * half + off:
                                           512 * (half + 1)],
                                        st[:, 512 * half + off:
                                           512 * (half + 1)], ACTF.Exp)
                            else:
                                nc.scalar.activation(pt[:], st[:], ACTF.Exp)
                            return pt

                        def ctx_stage_g(t2, g, pts=pts, ctx_ps=ctx_ps, p=p,
                                        nt2=nt2, j5=j5):
                            for half in range(2):
                                kb = 2 * t2 + half
                                off = (128 * (kb - 4 * j5)
                                       if kb >= 4 * j5 else 0)
                                nc.tensor.matmul(
                                    ctx_ps[g][:, off:512],
                                    vsc[p][g][:, kb, :],
                                    pts[t2][g][:, 512 * half + off:
                                               512 * (half + 1)],
                                    start=(kb == 0),
                                    stop=(kb == 2 * nt2 - 1))

                        # one-stage software pipeline, interleaved per g:
                        # [S_g0(t2), ctx_g0(t2-1), S_g1(t2), ctx_g1(t2-1)].
                        # S_g(t2)'s PSUM bank frees when exp(t2-1, g)
                        # completes, so the g0 work is compute-ready
                        # mid-stage and exp(t2, g0) starts the moment the
                        # ACT queue frees - ACT never idles.
                        for t2 in range(nt2):
                            pts[t2] = [None, None]
                            for g in range(2):
                                pts[t2][g] = s_stage_g(t2, g)
                                if t2 == 0 and g == 0 and pending_tail:
                                    # previous group's last ctx + evacuation
                                    # runs under this group's first S tiles
                                    pending_tail.pop()()
                                if t2 >= 1:
                                    ctx_stage_g(t2 - 1, g)
                            drain()

                        def group_tail(j5=j5, jh=jh, ctx_ps=ctx_ps,
                                       nt2=nt2, denS=denS, p=p,
                                       ctx_stage_g=ctx_stage_g):
                            for g in range(2):
                                ctx_stage_g(nt2 - 1, g)
                            # evacuate PSUM fast (frees the ctx banks for
                            # the next group); the reciprocal/divide runs
                            # later, overlapped under later compute
                            for g in range(2):
                                # [c, rc, m, 64jh+rr] <- ctx col (8rr + m)
                                nc.vector.tensor_copy(
                                    ctxP[p][64 * g:64 * (g + 1), j5 // 2, :,
                                            64 * jh:64 * (jh + 1)],
                                    ctx_ps[g][0:64, :].rearrange(
                                        "c (rr m) -> c m rr", m=8))
                                nc.vector.tensor_copy(
                                    denS[32 * (2 * jh + g):
                                         32 * (2 * jh + g) + 1, :],
                                    ctx_ps[g][64:65, :])

                        pending_tail.append(group_tail)

                        if jh == 0:
                            rc = j5 // 2
                            recbox = []

                            def mkrec(denS=denS, recbox=recbox):
                                recS = mp.tile([128, 512], F32, tag="recS",
                                               name="recS")
                                # custom-DVE fast reciprocal (~18 bits,
                                # ~5x cheaper than InstReciprocal); one
                                # partition-parallel call covers all four
                                # staged denominator rows
                                nc.vector.reciprocal_approx_fast(
                                    recS[:], denS[:])
                                recbox.append(recS)

                            def divide(jh2, p=p, rc=rc, recbox=recbox):
                                recS = recbox[0]
                                # rec rows for both g at partitions 0/32,
                                # then ONE PE matmul with the gsel selector
                                # broadcasts them to partitions 0-63 /
                                # 64-127 in PSUM
                                reck2 = reck2s[jh2]
                                for g in range(2):
                                    k4 = 2 * jh2 + g
                                    nc.vector.tensor_copy(
                                        reck2[32 * g:32 * g + 1, :],
                                        recS[32 * k4:32 * k4 + 1, :])
                                rbc_ps = psop.tile([128, 512], F32,
                                                   tag="psO", name="rbcps")
                                nc.tensor.matmul(
                                    rbc_ps[:], gsel[:], reck2[0:33, :],
                                    start=True, stop=True)
                                for g in range(2):
                                    dst = ctxP[p][64 * g:64 * (g + 1),
                                                  rc, :,
                                                  64 * jh2:64 * (jh2 + 1)]
                                    nc.vector.tensor_tensor(
                                        dst, dst,
                                        rbc_ps[64 * g:64 * (g + 1), :]
                                        .rearrange(
                                            "c (rr m) -> c m rr", m=8),
                                        ALU.mult)

                            # stagger the deferred chain so consecutive
                            # psO-bank users never wait on each other's
                            # evacuation inside the PE FIFO; for p=1 only
                            # 6 drain ticks remain after the rc=1 pair, so
                            # compress the countdowns to fire in-loop
                            if p == 0:
                                cds = (3, 3, 5, 7, 9)
                            else:
                                cds = (2, 2, 3, 4, 6)
                            pending.append([cds[0], mkrec])
                            pending.append([cds[1], lambda d=divide: d(0)])
                            pending.append([cds[2], lambda d=divide: d(1)])
                            pending.append(
                                [cds[3],
                                 lambda p=p, rc=rc: out_proj(p, rc, 0)])
                            pending.append(
                                [cds[4],
                                 lambda p=p, rc=rc: out_proj(p, rc, 1)])
                # latency-bound tail: keep the PE clock warm between the
                # dependency-chained final divides / out-projections
                warm(10)
                while pending_tail:
                    pending_tail.pop()()
                    warm(4)
                for item in pending:
                    item[1]()
                    warm(6)

    nc.compile()
    return nc


def _get_nc():
    key = "nc"
    if key not in _cache:
        _cache[key] = _build()
    return _cache[key]


def pack_in_maps(x, Wq, bq, Wk, bk, Wv, bv, Wo, bo):
    BF = ml_dtypes.bfloat16
    x = np.asarray(x, np.float32)
    WqT = np.ascontiguousarray(np.asarray(Wq, np.float32).T.astype(BF))
    WkT = np.ascontiguousarray(np.asarray(Wk, np.float32).T.astype(BF))
    WvT = np.ascontiguousarray(np.asarray(Wv, np.float32).T.astype(BF))
    # woTre[64g + d, m, o] = Wo[o, 512g + 64m + d]
    WoTre = np.ascontiguousarray(
        np.asarray(Wo, np.float32).T.reshape(2, 8, 64, E).transpose(0, 2, 1, 3)
        .reshape(128, 8, E).astype(BF))
    bqT = np.ascontiguousarray(np.asarray(bq, np.float32).reshape(8, 128).T)
    bkT8 = np.ascontiguousarray((np.asarray(bk, np.float32) / 8.0).reshape(8, 128).T)
    bvrow = np.asarray(bv, np.float32).reshape(1, E)
    borow = np.asarray(bo, np.float32).reshape(1, E)
    # n'-contiguous layout: every diagonal 128-block shares one
    # upper-triangle mask
    ii = np.arange(128)[:, None]
    cc = np.arange(128)[None, :]
    masks = np.where(ii <= cc, 0.0, NEG).astype(BF)
    ident = np.eye(128).astype(BF)
    gsel = np.zeros((33, 128), np.float32)
    gsel[0, 0:64] = 1.0
    gsel[32, 64:128] = 1.0
    gsel = gsel.astype(BF)

    in_maps = []
    for c in range(8):
        xTs = np.empty((E, R), BF)
        for p in range(2):
            h = 2 * c + p
            b_, mp_ = divmod(h, 8)
            xTs[:, RP * p:RP * (p + 1)] = x[b_, RP * mp_:RP * (mp_ + 1), :].T.astype(BF)
        in_maps.append({
            "xT": np.ascontiguousarray(xTs), "wqT": WqT, "wkT": WkT,
            "wvT": WvT, "woTre": WoTre, "bqT": bqT, "bkT8": bkT8,
            "bvrow": bvrow, "borow": borow, "masks": masks, "ident": ident,
            "gsel": gsel,
        })
    return in_maps


def unpack_out(results):
    out = np.empty((2, 2048, E), np.float32)
    for c in range(8):
        o = results[c]["out"]
        for p in range(2):
            h = 2 * c + p
            b_, mp_ = divmod(h, 8)
            out[b_, RP * mp_:RP * (mp_ + 1), :] = o[RP * p:RP * (p + 1), :]
    return out


def kernel(x, Wq, bq, Wk, bk, Wv, bv, Wo, bo):
    in_maps = pack_in_maps(x, Wq, bq, Wk, bk, Wv, bv, Wo, bo)
    nc = _get_nc()
    res = run_bass_kernel_spmd(nc, in_maps, core_ids=list(range(8)))
    return unpack_out(res.results)


# revision 31
# speedup vs baseline: 1.0676x; 1.0676x over previous
"""MicroHeadAttention Trainium2 kernel (8-core SPMD, data-parallel over
(batch, row-chunk) pairs).

Shapes (hardcoded): x (2, 2048, 1024), weights (1024, 1024), biases (1024,).
EMBED=1024, 16 heads in 2 blocks (g) of 8 micro-heads, head_dim 64.

Decomposition: the reference's "scramble" is a raw row-major reshape, so the
attention head (b, g, m') consumes exactly rows x[b, 256m':256(m'+1)] and
weight columns [512g:512(g+1)], reshaped (256, 512) -> (2048, 64) with
scrambled position n' = 8*row + m (m = 64-channel sub-block).  16 (b, m')
row-chunks across 8 cores = 2 per core; each chunk has g=0,1 -> 4 heads/core.

v3.1 schedule notes:
  - ACT (exp) is the bottleneck engine of the attention phase (~91us of
    exp at ~1.1us per [128,1024] tile); everything is arranged so ACT never
    waits: per-g S stages and ctx stages are interleaved
    [S_g0(t2+1), ctx_g0(t2), S_g1(t2+1), ctx_g1(t2)] so the g0 tiles of the
    next stage (whose PSUM banks free when exp(t2, g0) completes mid-stage)
    are compute-ready the moment the ACT queue frees up.
  - exp stays at [128,1024] per-g granularity: splitting it per half costs
    ~155ns/instruction of ACT overhead (+20us measured in v3).
  - all weight DMAs start up front from persistent tiles (single 2MB
    transfers); no pool-reuse dependencies anywhere.
  - the V projection for the second row-pair (p=1) is deferred into the
    early attention phase (PE slack under the ACT-bound cadence), shrinking
    the serial projection prefix.
  - softmax divide: gpsimd partition_broadcast expands the reciprocal rows
    and gpsimd tensor_tensor multiplies them into ctxP - no PE broadcast
    matmul, no PSUM evacuation, nothing on the (busy) DVE.
  - deferred out-proj / rbc emissions are staggered so consecutive users of
    the single psO bank never stall the PE FIFO; the final drain interleaves
    keep-warm matmuls on a spare PSUM bank so the latency-bound tail chain
    runs at 2.4GHz.
"""

import ml_dtypes
import numpy as np

import concourse.bass as bass
import concourse.mybir as mybir
from concourse import bacc
from concourse.tile import TileContext
from concourse.bass_utils import run_bass_kernel_spmd

F32 = mybir.dt.float32
BF16 = mybir.dt.bfloat16
DT_MM = BF16
NEG = -1e30
E = 1024
R = 512       # rows per core
RP = 256      # rows per pair
ALU = mybir.AluOpType
ACTF = mybir.ActivationFunctionType

_cache = {}


def _build():
    nc = bacc.Bacc()
    xT_d = nc.dram_tensor("xT", (E, R), DT_MM, kind="ExternalInput")
    wq_d = nc.dram_tensor("wqT", (E, E), DT_MM, kind="ExternalInput")
    wk_d = nc.dram_tensor("wkT", (E, E), DT_MM, kind="ExternalInput")
    wv_d = nc.dram_tensor("wvT", (E, E), DT_MM, kind="ExternalInput")
    wo_d = nc.dram_tensor("woTre", (128, 8, E), DT_MM, kind="ExternalInput")
    bq_d = nc.dram_tensor("bqT", (128, 8), F32, kind="ExternalInput")
    bk_d = nc.dram_tensor("bkT8", (128, 8), F32, kind="ExternalInput")
    bv_d = nc.dram_tensor("bvrow", (1, E), F32, kind="ExternalInput")
    bo_d = nc.dram_tensor("borow", (1, E), F32, kind="ExternalInput")
    masks_d = nc.dram_tensor("masks", (128, 128), DT_MM, kind="ExternalInput")
    ident_d = nc.dram_tensor("ident", (128, 128), DT_MM, kind="ExternalInput")
    gsel_d = nc.dram_tensor("gsel", (33, 128), DT_MM, kind="ExternalInput")
    out_d = nc.dram_tensor("out", (R, E), F32, kind="ExternalOutput")

    with TileContext(nc) as tc:
        with (
            tc.tile_pool(name="persist", bufs=1) as pp,
            tc.tile_pool(name="pt", bufs=4) as ptp,
            tc.tile_pool(name="misc", bufs=2) as mp,
            tc.tile_pool(name="outs", bufs=4) as osp,
            tc.tile_pool(name="dram", bufs=1, space="DRAM") as dp,
        ):
            # ---- persistent tiles ----
            bqT = pp.tile([128, 8], F32, tag="bqT", name="bqT")
            bkT8 = pp.tile([128, 8], F32, tag="bkT8", name="bkT8")
            # n'-contiguous layout: qsc/ksc/vsc columns are sorted by the
            # scrambled position n' = 8*rr + m, so causality is
            # block-triangular: k-blocks strictly below the diagonal are
            # fully visible and the single [128,128] upper-triangle mask
            # covers every diagonal block.
            masks = pp.tile([128, 128], DT_MM, tag="masks", name="masks")
            # dependency-free all-zeros warm operand (memset, no DMA)
            wrm = pp.tile([128, 512], DT_MM, tag="wrm", name="wrm")
            ident = pp.tile([128, 128], DT_MM, tag="ident", name="ident")
            gsel = pp.tile([33, 128], DT_MM, tag="gsel", name="gsel")
            # persistent rec-row staging (rows 1-31 stay zero so the 33-wide
            # gsel broadcast matmul never reads uninitialized SBUF)
            reck2s = [pp.tile([33, 512], DT_MM, tag=f"reck2{i}",
                              name=f"reck2{i}") for i in range(2)]
            qsc = pp.tile([128, 4096], DT_MM, tag="qsc", name="qsc")
            ksc = pp.tile([128, 4096], DT_MM, tag="ksc", name="ksc")
            vsc = [[pp.tile([128, 16, 65], DT_MM, tag=f"vsc{p}{g}", name=f"vsc{p}{g}")
                    for g in range(2)] for p in range(2)]
            # ctxP[p][c, rc, m, rr] : out-proj lhsT slices are contiguous
            # (FWL needs a single-stride stationary AP); with m-major ctx
            # columns the divide writes 64-contiguous runs into it
            ctxP = [pp.tile([128, 2, 8, 128], DT_MM, tag=f"ctxP{p}", name=f"ctxP{p}")
                    for p in range(2)]
            vtmp = dp.tile([2, 2, 2048, 64], DT_MM, tag="vtmp", name="vtmp")

            xt = pp.tile([128, 8, R], DT_MM, tag="xt", name="xt")
            wq = pp.tile([128, 8, E], DT_MM, tag="wq", name="wq")
            wk = pp.tile([128, 8, E], DT_MM, tag="wk", name="wk")
            wv = pp.tile([128, 8, E], DT_MM, tag="wv", name="wv")
            wo = pp.tile([128, 8, E], DT_MM, tag="wo", name="wo")
            vnat = [pp.tile([128, 2, E], DT_MM, tag=f"vnat{p}", name=f"vnat{p}")
                    for p in range(2)]
            bvr = pp.tile([1, E], F32, tag="bvr", name="bvr")
            bv_bc = pp.tile([128, E], F32, tag="bvbc", name="bvbc")
            bor = pp.tile([1, E], F32, tag="bor", name="bor")
            bo_bc = pp.tile([128, E], F32, tag="bobc", name="bobc")

            # masks/ident are host constants so the PE pre-warm only
            # waits on these two small DMAs
            nc.sync.dma_start(masks[:], masks_d[:])
            nc.sync.dma_start(ident[:], ident_d[:])
            nc.sync.dma_start(gsel[:], gsel_d[:])
            nc.sync.dma_start(bqT[:], bq_d[:])
            nc.sync.dma_start(bkT8[:], bk_d[:])
            nc.sync.dma_start(bvr[:], bv_d[:])
            nc.sync.dma_start(bor[:], bo_d[:])

            nc.sync.dma_start(xt[:], xT_d.rearrange("(ko ki) r -> ki ko r", ki=128))
            wv_v = wv_d.rearrange("(ko ki) o -> ki ko o", ki=128)
            # oc-major so V's first output half can start after 1 MB
            for oc in range(2):
                nc.sync.dma_start(wv[:, :, 512 * oc:512 * (oc + 1)],
                                  wv_v[:, :, 512 * oc:512 * (oc + 1)])
            nc.sync.dma_start(wq[:], wq_d.rearrange("(ko ki) o -> ki ko o", ki=128))
            nc.sync.dma_start(wk[:], wk_d.rearrange("(ko ki) o -> ki ko o", ki=128))
            nc.sync.dma_start(wo[:], wo_d[:])

            nc.gpsimd.partition_broadcast(bv_bc[:], bvr[:])
            nc.gpsimd.partition_broadcast(bo_bc[:], bor[:])
            ones16 = pp.tile([128, 16], F32, tag="ones16", name="ones16")
            nc.gpsimd.memset(ones16[:], 1.0)
            nc.vector.memset(wrm[:], 0.0)
            for i in range(2):
                nc.vector.memset(reck2s[i][:], 0.0)
            for p in range(2):
                for g in range(2):
                    nc.vector.tensor_copy(vsc[p][g][:, :, 64], ones16[:])

            def v_group(oc, rc, psp_, tag="psA"):
                p, half = rc // 2, rc % 2
                ps = psp_.tile([128, 512], F32, tag=tag, name="psA")
                for ki in range(8):
                    nc.tensor.matmul(
                        ps[:], xt[:, ki, 128 * rc:128 * (rc + 1)],
                        wv[:, ki, 512 * oc:512 * (oc + 1)],
                        start=(ki == 0), stop=(ki == 7))
                nc.vector.tensor_tensor(
                    vnat[p][:, half, 512 * oc:512 * (oc + 1)],
                    ps[:], bv_bc[:, 512 * oc:512 * (oc + 1)], ALU.add)

            def v_scramble(p):
                for g in range(2):
                    # n' = 8*(128h + 64rb + rr) + m = 1024h + 512rb
                    # + 8rr + m.  Split per (h, rb): the DMA balancer
                    # tops out at 3 dims.
                    dstv = vtmp[p, g].rearrange(
                        "(h rb rr m) d -> h rb rr m d", h=2, rb=2, m=8)
                    for h in range(2):
                        for rb in range(2):
                            srcs = vnat[p][64 * rb:64 * (rb + 1), h,
                                           512 * g:512 * (g + 1)]
                            nc.sync.dma_start(
                                dstv[h, rb],
                                srcs.rearrange("rr (m d) -> rr m d", m=8))
                for g in range(2):
                    # vsc partition index is the in-block m-major coord
                    # 16m + rr%16; vtmp rows are flat n' = 128 kb + 8 rr
                    # + m, so gather per m to keep the AP affine
                    src_v = vtmp[p, g].rearrange(
                        "(kb rr mm) d -> rr mm kb d", kb=16, rr=16)
                    for m in range(8):
                        nc.sync.dma_start(
                            vsc[p][g][16 * m:16 * (m + 1), :, 0:64],
                            src_v[:, m])

            with tc.tile_pool(name="ps1", bufs=5, space="PSUM") as psp, \
                 tc.tile_pool(name="pswarm", bufs=1, space="PSUM") as pwp:
                # ---- PE pre-warm: back-to-back matmuls on the mask tile
                # keep the HAM activity window busy while the input DMAs
                # stream, so the real matmuls start at 2.4 GHz.
                psw = pwp.tile([128, 512], F32, tag="psw", name="psw")
                for _ in range(24):
                    nc.tensor.matmul(psw[:], ident[:], wrm[:],
                                     start=True, stop=True)

                def qk_proj(w_tile, bias_tile, scale, dst):
                    for t in range(8):
                        ps = psp.tile([128, 512], F32, tag="psA", name="psA")
                        for ki in range(8):
                            nc.tensor.matmul(
                                ps[:], w_tile[:, ki, 128 * t:128 * (t + 1)],
                                xt[:, ki, :], start=(ki == 0), stop=(ki == 7))
                        g, u = t // 4, t % 4
                        for mh in range(2):
                            mmv = 2 * u + mh
                            # position-sorted 128-blocks (b = rr//16),
                            # m-major inside the block: col = 2048 pp
                            # + 128 b + 16 m + rr%16 - head m's channels
                            # land in contiguous 16-element runs
                            dest = dst.rearrange(
                                "c (pp b mm rrlo) -> c pp b mm rrlo",
                                pp=2, b=16, mm=8, rrlo=16)[
                                64 * g:64 * (g + 1), :, :, mmv, :]
                            src = ps[64 * mh:64 * (mh + 1), :].rearrange(
                                "c (pp b rrlo) -> c pp b rrlo",
                                pp=2, b=16)
                            # VectorE lanes are partition-locked: it can only
                            # take the copies whose src/dst partition ranges
                            # line up (g == mh); ScalarE handles the crossed
                            # ones.
                            if mh != g:
                                nc.scalar.activation(
                                    dest, src, ACTF.Identity,
                                    bias=bias_tile[64 * mh:64 * (mh + 1), t:t + 1],
                                    scale=scale)
                            else:
                                nc.vector.tensor_scalar(
                                    out=dest, in0=src, scalar1=scale,
                                    scalar2=bias_tile[64 * mh:64 * (mh + 1), t:t + 1],
                                    op0=ALU.mult, op1=ALU.add)

                # V projection for the first row-pair only; p=1 is deferred
                # into the attention phase (PE slack under the ACT-bound
                # cadence).
                for oc in range(2):
                    for rc in range(2):
                        v_group(oc, rc, psp)
                v_scramble(0)

                qk_proj(wq, bqT, 1.0, qsc)
                qk_proj(wk, bkT8, 0.125, ksc)

            # ---- attention + interleaved output projection ----
            with tc.tile_pool(name="psS", bufs=2, space="PSUM") as pssp, \
                 tc.tile_pool(name="psctx", bufs=2, space="PSUM") as pcp, \
                 tc.tile_pool(name="psO", bufs=1, space="PSUM") as psop, \
                 tc.tile_pool(name="pswarm2", bufs=1, space="PSUM") as pwp2:

                warm_ps = pwp2.tile([128, 512], F32, tag="warm", name="warm")

                def warm(n):
                    for _ in range(n):
                        nc.tensor.matmul(warm_ps[:], ident[:], wrm[:],
                                         start=True, stop=True)

                def out_proj(p, rc, oc):
                    ps = psop.tile([128, 512], F32, tag="psO", name="psO")
                    for mmv in range(8):
                        nc.tensor.matmul(
                            ps[:],
                            ctxP[p][:, rc, mmv, :],
                            wo[:, mmv, 512 * oc:512 * (oc + 1)],
                            start=(mmv == 0), stop=(mmv == 7))
                    outsb = osp.tile([128, 512], F32, tag="outsb",
                                     name="outsb")
                    nc.vector.tensor_tensor(
                        outsb[:], ps[:],
                        bo_bc[:, 512 * oc:512 * (oc + 1)], ALU.add)
                    nc.sync.dma_start(
                        out_d[RP * p + 128 * rc:RP * p + 128 * (rc + 1),
                              512 * oc:512 * (oc + 1)],
                        outsb[:])

                # deferred emissions: [countdown_in_t2_steps, fn]
                pending = []

                def drain():
                    for item in pending[:]:
                        item[0] -= 1
                        if item[0] <= 0:
                            pending.remove(item)
                            item[1]()

                # V projection p=1 + scramble, spread over the early
                # attention stages (psO bank is idle there; out-proj
                # deferrals only begin after the first rc completes)
                for i, (oc, rc) in enumerate([(0, 2), (0, 3), (1, 2), (1, 3)]):
                    pending.append(
                        [1 + 3 * i,
                         lambda oc=oc, rc=rc: v_group(oc, rc, psop, "psO")])
                pending.append([13, lambda: v_scramble(1)])

                pending_tail = []
                for p in range(2):
                    denS = None
                    # descending j5: pairs (3,2) then (1,0), so each
                    # pair-completion divide is covered by a LONG next
                    # group (the p-transition lands on nt2=8, not 2)
                    for j5 in (3, 2, 1, 0):
                        jh = j5 % 2
                        if jh == 1:
                            # 4 denominator rows (jh, g) staged on
                            # separate partitions: one partition-parallel
                            # reciprocal per rc instead of four
                            denS = mp.tile([128, 512], F32, tag="denS",
                                           name="denS")
                        nt2 = 2 * (j5 + 1)   # pairs of 128-wide k blocks
                        ctx_ps = [pcp.tile([65, 512], F32, tag="ctxps",
                                           name="ctxps")
                                  for _ in range(2)]
                        pts = [None] * nt2

                        def s_stage_g(t2, g):
                            # S for one g: two 512-col halves into one
                            # [128,1024] PSUM tile; diagonal k-blocks only
                            # stream the causal q range (cols >= the
                            # block's position offset) and get the shared
                            # [128,128] triangle mask via an N=128
                            # identity matmul, emitted after both S halves
                            # so the same-bank accumulation never waits on
                            # an undrained S write.
                            st = pssp.tile([128, 1024], F32, tag="st",
                                           name="st")
                            pt = ptp.tile([128, 1024], DT_MM, tag="pt",
                                          name="pt")
                            diag = t2 >= 2 * j5
                            for half in range(2):
                                kb = 2 * t2 + half
                                off = 128 * (kb - 4 * j5) if diag else 0
                                nc.tensor.matmul(
                                    st[:, 512 * half + off:
                                       512 * (half + 1)],
                                    ksc[64 * g:64 * (g + 1),
                                        2048 * p + 128 * kb:
                                        2048 * p + 128 * (kb + 1)],
                                    qsc[64 * g:64 * (g + 1),
                                        2048 * p + 512 * j5 + off:
                                        2048 * p + 512 * (j5 + 1)],
                                    start=True, stop=not diag)
                            if diag:
                                for half in range(2):
                                    kb = 2 * t2 + half
                                    off = 128 * (kb - 4 * j5)
                                    nc.tensor.matmul(
                                        st[:, 512 * half + off:
                                           512 * half + off + 128],
                                        ident[:], masks[:],
                                        start=False, stop=True)
                                for half in range(2):
                                    off = 128 * (2 * t2 + half - 4 * j5)
                                    nc.scalar.activation(
                                        pt[:, 512 * half + off:
                                           512 * (half + 1)],
                                        st[:, 512 * half + off:
                                           512 * (half + 1)], ACTF.Exp)
                            else:
                                nc.scalar.activation(pt[:], st[:], ACTF.Exp)
                            return pt

                        def ctx_stage_g(t2, g, pts=pts, ctx_ps=ctx_ps, p=p,
                                        nt2=nt2, j5=j5):
                            for half in range(2):
                                kb = 2 * t2 + half
                                off = (128 * (kb - 4 * j5)
                                       if kb >= 4 * j5 else 0)
                                nc.tensor.matmul(
                                    ctx_ps[g][:, off:512],
                                    vsc[p][g][:, kb, :],
                                    pts[t2][g][:, 512 * half + off:
                                               512 * (half + 1)],
                                    start=(kb == 0),
                                    stop=(kb == 2 * nt2 - 1))

                        # one-stage software pipeline, interleaved per g:
                        # [S_g0(t2), ctx_g0(t2-1), S_g1(t2), ctx_g1(t2-1)].
                        # S_g(t2)'s PSUM bank frees when exp(t2-1, g)
                        # completes, so the g0 work is compute-ready
                        # mid-stage and exp(t2, g0) starts the moment the
                        # ACT queue frees - ACT never idles.
                        for t2 in range(nt2):
                            pts[t2] = [None, None]
                            for g in range(2):
                                pts[t2][g] = s_stage_g(t2, g)
                                if t2 == 0 and g == 0 and pending_tail:
                                    # previous group's last ctx + evacuation
                                    # runs under this group's first S tiles
                                    pending_tail.pop()()
                                if t2 >= 1:
                                    ctx_stage_g(t2 - 1, g)
                            drain()

                        def group_tail(j5=j5, jh=jh, ctx_ps=ctx_ps,
                                       nt2=nt2, denS=denS, p=p,
                                       ctx_stage_g=ctx_stage_g):
                            for g in range(2):
                                ctx_stage_g(nt2 - 1, g)
                            # evacuate PSUM fast (frees the ctx banks for
                            # the next group); the reciprocal/divide runs
                            # later, overlapped under later compute
                            for g in range(2):
                                # [c, rc, m, 64jh + 16b + rrlo] <- ctx col
                                # (128b + 16m + rrlo)
                                nc.vector.tensor_copy(
                                    ctxP[p][64 * g:64 * (g + 1), j5 // 2, :,
                                            64 * jh:64 * (jh + 1)]
                                    .rearrange("c m (b rr) -> c m b rr",
                                               b=4),
                                    ctx_ps[g][0:64, :].rearrange(
                                        "c (b m rr) -> c m b rr",
                                        b=4, m=8))
                                nc.vector.tensor_copy(
                                    denS[32 * (2 * jh + g):
                                         32 * (2 * jh + g) + 1, :],
                                    ctx_ps[g][64:65, :])

                        pending_tail.append(group_tail)

                        if jh == 0:
                            rc = j5 // 2
                            recbox = []

                            def mkrec(denS=denS, recbox=recbox):
                                recS = mp.tile([128, 512], F32, tag="recS",
                                               name="recS")
                                # custom-DVE fast reciprocal (~18 bits,
                                # ~5x cheaper than InstReciprocal); one
                                # partition-parallel call covers all four
                                # staged denominator rows
                                nc.vector.reciprocal_approx_fast(
                                    recS[:], denS[:])
                                recbox.append(recS)

                            def divide(jh2, p=p, rc=rc, recbox=recbox):
                                recS = recbox[0]
                                # rec rows for both g at partitions 0/32,
                                # then ONE PE matmul with the gsel selector
                                # broadcasts them to partitions 0-63 /
                                # 64-127 in PSUM
                                reck2 = reck2s[jh2]
                                for g in range(2):
                                    k4 = 2 * jh2 + g
                                    nc.vector.tensor_copy(
                                        reck2[32 * g:32 * g + 1, :],
                                        recS[32 * k4:32 * k4 + 1, :])
                                rbc_ps = psop.tile([128, 512], F32,
                                                   tag="psO", name="rbcps")
                                nc.tensor.matmul(
                                    rbc_ps[:], gsel[:], reck2[0:33, :],
                                    start=True, stop=True)
                                for g in range(2):
                                    dst = ctxP[p][64 * g:64 * (g + 1),
                                                  rc, :,
                                                  64 * jh2:64 * (jh2 + 1)]
                                    nc.vector.tensor_tensor(
                                        dst.rearrange(
                                            "c m (b rr) -> c m b rr", b=4),
                                        dst.rearrange(
                                            "c m (b rr) -> c m b rr", b=4),
                                        rbc_ps[64 * g:64 * (g + 1), :]
                                        .rearrange(
                                            "c (b m rr) -> c m b rr",
                                            b=4, m=8),
                                        ALU.mult)

                            # stagger the deferred chain so consecutive
                            # psO-bank users never wait on each other's
                            # evacuation inside the PE FIFO; for p=1 only
                            # 6 drain ticks remain after the rc=1 pair, so
                            # compress the countdowns to fire in-loop
                            if p == 0:
                                cds = (3, 3, 5, 7, 9)
                            else:
                                cds = (2, 2, 3, 4, 6)
                            pending.append([cds[0], mkrec])
                            pending.append([cds[1], lambda d=divide: d(0)])
                            pending.append([cds[2], lambda d=divide: d(1)])
                            pending.append(
                                [cds[3],
                                 lambda p=p, rc=rc: out_proj(p, rc, 0)])
                            pending.append(
                                [cds[4],
                                 lambda p=p, rc=rc: out_proj(p, rc, 1)])
                # latency-bound tail: keep the PE clock warm between the
                # dependency-chained final divides / out-projections
                warm(10)
                while pending_tail:
                    pending_tail.pop()()
                    warm(4)
                for item in pending:
                    item[1]()
                    warm(6)

    nc.compile()
    return nc


def _get_nc():
    key = "nc"
    if key not in _cache:
        _cache[key] = _build()
    return _cache[key]


def pack_in_maps(x, Wq, bq, Wk, bk, Wv, bv, Wo, bo):
    BF = ml_dtypes.bfloat16
    x = np.asarray(x, np.float32)
    WqT = np.ascontiguousarray(np.asarray(Wq, np.float32).T.astype(BF))
    WkT = np.ascontiguousarray(np.asarray(Wk, np.float32).T.astype(BF))
    WvT = np.ascontiguousarray(np.asarray(Wv, np.float32).T.astype(BF))
    # woTre[64g + d, m, o] = Wo[o, 512g + 64m + d]
    WoTre = np.ascontiguousarray(
        np.asarray(Wo, np.float32).T.reshape(2, 8, 64, E).transpose(0, 2, 1, 3)
        .reshape(128, 8, E).astype(BF))
    bqT = np.ascontiguousarray(np.asarray(bq, np.float32).reshape(8, 128).T)
    bkT8 = np.ascontiguousarray((np.asarray(bk, np.float32) / 8.0).reshape(8, 128).T)
    bvrow = np.asarray(bv, np.float32).reshape(1, E)
    borow = np.asarray(bo, np.float32).reshape(1, E)
    # position-sorted 128-blocks, m-major in-block: index i = 16m + rr%16
    # has in-block position 8*(i%16) + i//16; one mask covers every
    # diagonal block
    ii = np.arange(128)[:, None]
    cc = np.arange(128)[None, :]
    pos_k = 8 * (ii % 16) + ii // 16
    pos_q = 8 * (cc % 16) + cc // 16
    masks = np.where(pos_k <= pos_q, 0.0, NEG).astype(BF)
    ident = np.eye(128).astype(BF)
    gsel = np.zeros((33, 128), np.float32)
    gsel[0, 0:64] = 1.0
    gsel[32, 64:128] = 1.0
    gsel = gsel.astype(BF)

    in_maps = []
    for c in range(8):
        xTs = np.empty((E, R), BF)
        for p in range(2):
            h = 2 * c + p
            b_, mp_ = divmod(h, 8)
            xTs[:, RP * p:RP * (p + 1)] = x[b_, RP * mp_:RP * (mp_ + 1), :].T.astype(BF)
        in_maps.append({
            "xT": np.ascontiguousarray(xTs), "wqT": WqT, "wkT": WkT,
            "wvT": WvT, "woTre": WoTre, "bqT": bqT, "bkT8": bkT8,
            "bvrow": bvrow, "borow": borow, "masks": masks, "ident": ident,
            "gsel": gsel,
        })
    return in_maps


def unpack_out(results):
    out = np.empty((2, 2048, E), np.float32)
    for c in range(8):
        o = results[c]["out"]
        for p in range(2):
            h = 2 * c + p
            b_, mp_ = divmod(h, 8)
            out[b_, RP * mp_:RP * (mp_ + 1), :] = o[RP * p:RP * (p + 1), :]
    return out


def kernel(x, Wq, bq, Wk, bk, Wv, bv, Wo, bo):
    in_maps = pack_in_maps(x, Wq, bq, Wk, bk, Wv, bv, Wo, bo)
    nc = _get_nc()
    res = run_bass_kernel_spmd(nc, in_maps, core_ids=list(range(8)))
    return unpack_out(res.results)


# revision 33
# speedup vs baseline: 1.1262x; 1.0549x over previous
"""MicroHeadAttention Trainium2 kernel (8-core SPMD, data-parallel over
(batch, row-chunk) pairs).

Shapes (hardcoded): x (2, 2048, 1024), weights (1024, 1024), biases (1024,).
EMBED=1024, 16 heads in 2 blocks (g) of 8 micro-heads, head_dim 64.

Decomposition: the reference's "scramble" is a raw row-major reshape, so the
attention head (b, g, m') consumes exactly rows x[b, 256m':256(m'+1)] and
weight columns [512g:512(g+1)], reshaped (256, 512) -> (2048, 64) with
scrambled position n' = 8*row + m (m = 64-channel sub-block).  16 (b, m')
row-chunks across 8 cores = 2 per core; each chunk has g=0,1 -> 4 heads/core.

v3.1 schedule notes:
  - ACT (exp) is the bottleneck engine of the attention phase (~91us of
    exp at ~1.1us per [128,1024] tile); everything is arranged so ACT never
    waits: per-g S stages and ctx stages are interleaved
    [S_g0(t2+1), ctx_g0(t2), S_g1(t2+1), ctx_g1(t2)] so the g0 tiles of the
    next stage (whose PSUM banks free when exp(t2, g0) completes mid-stage)
    are compute-ready the moment the ACT queue frees up.
  - exp stays at [128,1024] per-g granularity: splitting it per half costs
    ~155ns/instruction of ACT overhead (+20us measured in v3).
  - all weight DMAs start up front from persistent tiles (single 2MB
    transfers); no pool-reuse dependencies anywhere.
  - the V projection for the second row-pair (p=1) is deferred into the
    early attention phase (PE slack under the ACT-bound cadence), shrinking
    the serial projection prefix.
  - softmax divide: gpsimd partition_broadcast expands the reciprocal rows
    and gpsimd tensor_tensor multiplies them into ctxP - no PE broadcast
    matmul, no PSUM evacuation, nothing on the (busy) DVE.
  - deferred out-proj / rbc emissions are staggered so consecutive users of
    the single psO bank never stall the PE FIFO; the final drain interleaves
    keep-warm matmuls on a spare PSUM bank so the latency-bound tail chain
    runs at 2.4GHz.
"""

import ml_dtypes
import numpy as np

import concourse.bass as bass
import concourse.mybir as mybir
from concourse import bacc
from concourse.tile import TileContext
from concourse.bass_utils import run_bass_kernel_spmd

F32 = mybir.dt.float32
BF16 = mybir.dt.bfloat16
DT_MM = BF16
NEG = -1e30
E = 1024
R = 512       # rows per core
RP = 256      # rows per pair
ALU = mybir.AluOpType
ACTF = mybir.ActivationFunctionType

_cache = {}


def _build():
    nc = bacc.Bacc()
    xT_d = nc.dram_tensor("xT", (E, R), DT_MM, kind="ExternalInput")
    wq_d = nc.dram_tensor("wqT", (E, E), DT_MM, kind="ExternalInput")
    wk_d = nc.dram_tensor("wkT", (E, E), DT_MM, kind="ExternalInput")
    wv_d = nc.dram_tensor("wvT", (E, E), DT_MM, kind="ExternalInput")
    wo_d = nc.dram_tensor("woTre", (128, 8, E), DT_MM, kind="ExternalInput")
    bq_d = nc.dram_tensor("bqT", (128, 8), F32, kind="ExternalInput")
    bk_d = nc.dram_tensor("bkT8", (128, 8), F32, kind="ExternalInput")
    bv_d = nc.dram_tensor("bvrow", (1, E), F32, kind="ExternalInput")
    bo_d = nc.dram_tensor("borow", (1, E), F32, kind="ExternalInput")
    masks_d = nc.dram_tensor("masks", (128, 128), DT_MM, kind="ExternalInput")
    ident_d = nc.dram_tensor("ident", (128, 128), DT_MM, kind="ExternalInput")
    gsel_d = nc.dram_tensor("gsel", (33, 128), DT_MM, kind="ExternalInput")
    out_d = nc.dram_tensor("out", (R, E), F32, kind="ExternalOutput")

    with TileContext(nc) as tc:
        with (
            tc.tile_pool(name="persist", bufs=1) as pp,
            tc.tile_pool(name="pt", bufs=4) as ptp,
            tc.tile_pool(name="misc", bufs=2) as mp,
            tc.tile_pool(name="outs", bufs=4) as osp,
            tc.tile_pool(name="dram", bufs=1, space="DRAM") as dp,
        ):
            # ---- persistent tiles ----
            bqT = pp.tile([128, 8], F32, tag="bqT", name="bqT")
            bkT8 = pp.tile([128, 8], F32, tag="bkT8", name="bkT8")
            # n'-contiguous layout: qsc/ksc/vsc columns are sorted by the
            # scrambled position n' = 8*rr + m, so causality is
            # block-triangular: k-blocks strictly below the diagonal are
            # fully visible and the single [128,128] upper-triangle mask
            # covers every diagonal block.
            masks = pp.tile([128, 128], DT_MM, tag="masks", name="masks")
            # dependency-free all-zeros warm operand (memset, no DMA)
            wrm = pp.tile([128, 512], DT_MM, tag="wrm", name="wrm")
            ident = pp.tile([128, 128], DT_MM, tag="ident", name="ident")
            gsel = pp.tile([33, 128], DT_MM, tag="gsel", name="gsel")
            # persistent rec-row staging (rows 1-31 stay zero so the 33-wide
            # gsel broadcast matmul never reads uninitialized SBUF)
            reck2s = [pp.tile([33, 512], DT_MM, tag=f"reck2{i}",
                              name=f"reck2{i}") for i in range(2)]
            qsc = pp.tile([128, 4096], DT_MM, tag="qsc", name="qsc")
            ksc = pp.tile([128, 4096], DT_MM, tag="ksc", name="ksc")
            vsc = [[pp.tile([128, 16, 65], DT_MM, tag=f"vsc{p}{g}", name=f"vsc{p}{g}")
                    for g in range(2)] for p in range(2)]
            # ctxP[p][c, rc, m, rr] : out-proj lhsT slices are contiguous
            # (FWL needs a single-stride stationary AP); with m-major ctx
            # columns the divide writes 64-contiguous runs into it
            ctxP = [pp.tile([128, 2, 8, 128], DT_MM, tag=f"ctxP{p}", name=f"ctxP{p}")
                    for p in range(2)]
            vtmp = dp.tile([2, 2, 2048, 64], DT_MM, tag="vtmp", name="vtmp")

            xt = pp.tile([128, 8, R], DT_MM, tag="xt", name="xt")
            wq = pp.tile([128, 8, E], DT_MM, tag="wq", name="wq")
            wk = pp.tile([128, 8, E], DT_MM, tag="wk", name="wk")
            wv = pp.tile([128, 8, E], DT_MM, tag="wv", name="wv")
            wo = pp.tile([128, 8, E], DT_MM, tag="wo", name="wo")
            vnat = [pp.tile([128, 2, E], DT_MM, tag=f"vnat{p}", name=f"vnat{p}")
                    for p in range(2)]
            bvr = pp.tile([1, E], F32, tag="bvr", name="bvr")
            bv_bc = pp.tile([128, E], F32, tag="bvbc", name="bvbc")
            bor = pp.tile([1, E], F32, tag="bor", name="bor")
            bo_bc = pp.tile([128, E], F32, tag="bobc", name="bobc")

            # masks/ident are host constants so the PE pre-warm only
            # waits on these two small DMAs
            nc.sync.dma_start(masks[:], masks_d[:])
            nc.sync.dma_start(ident[:], ident_d[:])
            nc.sync.dma_start(gsel[:], gsel_d[:])
            nc.sync.dma_start(bqT[:], bq_d[:])
            nc.sync.dma_start(bkT8[:], bk_d[:])
            nc.sync.dma_start(bvr[:], bv_d[:])
            nc.sync.dma_start(bor[:], bo_d[:])

            nc.sync.dma_start(xt[:], xT_d.rearrange("(ko ki) r -> ki ko r", ki=128))
            wv_v = wv_d.rearrange("(ko ki) o -> ki ko o", ki=128)
            # oc-major so V's first output half can start after 1 MB
            for oc in range(2):
                nc.sync.dma_start(wv[:, :, 512 * oc:512 * (oc + 1)],
                                  wv_v[:, :, 512 * oc:512 * (oc + 1)])
            nc.sync.dma_start(wq[:], wq_d.rearrange("(ko ki) o -> ki ko o", ki=128))
            nc.sync.dma_start(wk[:], wk_d.rearrange("(ko ki) o -> ki ko o", ki=128))
            nc.sync.dma_start(wo[:], wo_d[:])

            nc.gpsimd.partition_broadcast(bv_bc[:], bvr[:])
            nc.gpsimd.partition_broadcast(bo_bc[:], bor[:])
            ones16 = pp.tile([128, 16], F32, tag="ones16", name="ones16")
            nc.gpsimd.memset(ones16[:], 1.0)
            nc.vector.memset(wrm[:], 0.0)
            for i in range(2):
                nc.vector.memset(reck2s[i][:], 0.0)
            for p in range(2):
                for g in range(2):
                    nc.vector.tensor_copy(vsc[p][g][:, :, 64], ones16[:])

            def v_group(oc, rc, psp_, tag="psA"):
                p, half = rc // 2, rc % 2
                ps = psp_.tile([128, 512], F32, tag=tag, name="psA")
                for ki in range(8):
                    nc.tensor.matmul(
                        ps[:], xt[:, ki, 128 * rc:128 * (rc + 1)],
                        wv[:, ki, 512 * oc:512 * (oc + 1)],
                        start=(ki == 0), stop=(ki == 7))
                nc.vector.tensor_tensor(
                    vnat[p][:, half, 512 * oc:512 * (oc + 1)],
                    ps[:], bv_bc[:, 512 * oc:512 * (oc + 1)], ALU.add)

            def v_scramble(p):
                for g in range(2):
                    # n' = 8*(128h + 64rb + rr) + m = 1024h + 512rb
                    # + 8rr + m.  Split per (h, rb): the DMA balancer
                    # tops out at 3 dims.
                    dstv = vtmp[p, g].rearrange(
                        "(h rb rr m) d -> h rb rr m d", h=2, rb=2, m=8)
                    for h in range(2):
                        for rb in range(2):
                            srcs = vnat[p][64 * rb:64 * (rb + 1), h,
                                           512 * g:512 * (g + 1)]
                            nc.sync.dma_start(
                                dstv[h, rb],
                                srcs.rearrange("rr (m d) -> rr m d", m=8))
                for g in range(2):
                    # vsc partition index is the in-block m-major coord
                    # 16m + rr%16; vtmp rows are flat n' = 128 kb + 8 rr
                    # + m, so gather per m to keep the AP affine
                    src_v = vtmp[p, g].rearrange(
                        "(kb rr mm) d -> rr mm kb d", kb=16, rr=16)
                    for m in range(8):
                        nc.sync.dma_start(
                            vsc[p][g][16 * m:16 * (m + 1), :, 0:64],
                            src_v[:, m])

            with tc.tile_pool(name="ps1", bufs=5, space="PSUM") as psp, \
                 tc.tile_pool(name="pswarm", bufs=1, space="PSUM") as pwp:
                # ---- PE pre-warm: back-to-back matmuls on the mask tile
                # keep the HAM activity window busy while the input DMAs
                # stream, so the real matmuls start at 2.4 GHz.
                psw = pwp.tile([128, 512], F32, tag="psw", name="psw")
                for _ in range(24):
                    nc.tensor.matmul(psw[:], ident[:], wrm[:],
                                     start=True, stop=True)

                def qk_proj(w_tile, bias_tile, scale, dst):
                    for t in range(8):
                        ps = psp.tile([128, 512], F32, tag="psA", name="psA")
                        for ki in range(8):
                            nc.tensor.matmul(
                                ps[:], w_tile[:, ki, 128 * t:128 * (t + 1)],
                                xt[:, ki, :], start=(ki == 0), stop=(ki == 7))
                        g, u = t // 4, t % 4
                        for mh in range(2):
                            mmv = 2 * u + mh
                            # position-sorted 128-blocks (b = rr//16),
                            # m-major inside the block: col = 2048 pp
                            # + 128 b + 16 m + rr%16 - head m's channels
                            # land in contiguous 16-element runs
                            dest = dst.rearrange(
                                "c (pp b mm rrlo) -> c pp b mm rrlo",
                                pp=2, b=16, mm=8, rrlo=16)[
                                64 * g:64 * (g + 1), :, :, mmv, :]
                            src = ps[64 * mh:64 * (mh + 1), :].rearrange(
                                "c (pp b rrlo) -> c pp b rrlo",
                                pp=2, b=16)
                            # VectorE lanes are partition-locked: it can only
                            # take the copies whose src/dst partition ranges
                            # line up (g == mh); ScalarE handles the crossed
                            # ones.
                            if mh != g:
                                nc.scalar.activation(
                                    dest, src, ACTF.Identity,
                                    bias=bias_tile[64 * mh:64 * (mh + 1), t:t + 1],
                                    scale=scale)
                            else:
                                nc.vector.tensor_scalar(
                                    out=dest, in0=src, scalar1=scale,
                                    scalar2=bias_tile[64 * mh:64 * (mh + 1), t:t + 1],
                                    op0=ALU.mult, op1=ALU.add)

                # V projection for the first row-pair only; p=1 is deferred
                # into the attention phase (PE slack under the ACT-bound
                # cadence).
                for oc in range(2):
                    for rc in range(2):
                        v_group(oc, rc, psp)
                v_scramble(0)

                qk_proj(wq, bqT, 1.0, qsc)
                qk_proj(wk, bkT8, 0.125, ksc)

            # ---- attention + interleaved output projection ----
            with tc.tile_pool(name="psS", bufs=2, space="PSUM") as pssp, \
                 tc.tile_pool(name="psctx", bufs=2, space="PSUM") as pcp, \
                 tc.tile_pool(name="psO", bufs=1, space="PSUM") as psop, \
                 tc.tile_pool(name="pswarm2", bufs=1, space="PSUM") as pwp2:

                warm_ps = pwp2.tile([128, 512], F32, tag="warm", name="warm")

                def warm(n):
                    for _ in range(n):
                        nc.tensor.matmul(warm_ps[:], ident[:], wrm[:],
                                         start=True, stop=True)

                def out_proj(p, rc, oc):
                    ps = psop.tile([128, 512], F32, tag="psO", name="psO")
                    for mmv in range(8):
                        nc.tensor.matmul(
                            ps[:],
                            ctxP[p][:, rc, mmv, :],
                            wo[:, mmv, 512 * oc:512 * (oc + 1)],
                            start=(mmv == 0), stop=(mmv == 7))
                    outsb = osp.tile([128, 512], F32, tag="outsb",
                                     name="outsb")
                    nc.vector.tensor_tensor(
                        outsb[:], ps[:],
                        bo_bc[:, 512 * oc:512 * (oc + 1)], ALU.add)
                    nc.sync.dma_start(
                        out_d[RP * p + 128 * rc:RP * p + 128 * (rc + 1),
                              512 * oc:512 * (oc + 1)],
                        outsb[:])

                # deferred emissions: [countdown_in_t2_steps, fn]
                pending = []

                def drain():
                    for item in pending[:]:
                        item[0] -= 1
                        if item[0] <= 0:
                            pending.remove(item)
                            item[1]()

                # V projection p=1 + scramble, spread over the early
                # attention stages (psO bank is idle there; out-proj
                # deferrals only begin after the first rc completes)
                for i, (oc, rc) in enumerate([(0, 2), (0, 3), (1, 2), (1, 3)]):
                    pending.append(
                        [1 + 3 * i,
                         lambda oc=oc, rc=rc: v_group(oc, rc, psop, "psO")])
                pending.append([13, lambda: v_scramble(1)])

                pending_tail = []
                for p in range(2):
                    denS = None
                    # descending j5: pairs (3,2) then (1,0), so each
                    # pair-completion divide is covered by a LONG next
                    # group (the p-transition lands on nt2=8, not 2)
                    for j5 in (3, 2, 1, 0):
                        jh = j5 % 2
                        if jh == 1:
                            # 4 denominator rows (jh, g) staged on
                            # separate partitions: one partition-parallel
                            # reciprocal per rc instead of four
                            denS = mp.tile([128, 512], F32, tag="denS",
                                           name="denS")
                        nt2 = 2 * (j5 + 1)   # pairs of 128-wide k blocks
                        ctx_ps = [pcp.tile([65, 512], F32, tag="ctxps",
                                           name="ctxps")
                                  for _ in range(2)]
                        pts = [None] * nt2

                        def s_stage_g(t2, g):
                            # S for one g: two 512-col halves into one
                            # [128,1024] PSUM tile; diagonal k-blocks only
                            # stream the causal q range (cols >= the
                            # block's position offset) and get the shared
                            # [128,128] triangle mask via an N=128
                            # identity matmul, emitted after both S halves
                            # so the same-bank accumulation never waits on
                            # an undrained S write.
                            st = pssp.tile([128, 1024], F32, tag="st",
                                           name="st")
                            pt = ptp.tile([128, 1024], DT_MM, tag="pt",
                                          name="pt")
                            diag = t2 >= 2 * j5
                            for half in range(2):
                                kb = 2 * t2 + half
                                off = 128 * (kb - 4 * j5) if diag else 0
                                nc.tensor.matmul(
                                    st[:, 512 * half + off:
                                       512 * (half + 1)],
                                    ksc[64 * g:64 * (g + 1),
                                        2048 * p + 128 * kb:
                                        2048 * p + 128 * (kb + 1)],
                                    qsc[64 * g:64 * (g + 1),
                                        2048 * p + 512 * j5 + off:
                                        2048 * p + 512 * (j5 + 1)],
                                    start=True, stop=not diag)
                            if diag:
                                for half in range(2):
                                    kb = 2 * t2 + half
                                    off = 128 * (kb - 4 * j5)
                                    nc.tensor.matmul(
                                        st[:, 512 * half + off:
                                           512 * half + off + 128],
                                        ident[:], masks[:],
                                        start=False, stop=True)
                                for half in range(2):
                                    off = 128 * (2 * t2 + half - 4 * j5)
                                    nc.scalar.activation(
                                        pt[:, 512 * half + off:
                                           512 * (half + 1)],
                                        st[:, 512 * half + off:
                                           512 * (half + 1)], ACTF.Exp)
                            else:
                                nc.scalar.activation(pt[:], st[:], ACTF.Exp)
                            return pt

                        def ctx_stage_g(t2, g, pts=pts, ctx_ps=ctx_ps, p=p,
                                        nt2=nt2, j5=j5):
                            for half in range(2):
                                kb = 2 * t2 + half
                                off = (128 * (kb - 4 * j5)
                                       if kb >= 4 * j5 else 0)
                                nc.tensor.matmul(
                                    ctx_ps[g][:, off:512],
                                    vsc[p][g][:, kb, :],
                                    pts[t2][g][:, 512 * half + off:
                                               512 * (half + 1)],
                                    start=(kb == 0),
                                    stop=(kb == 2 * nt2 - 1))

                        # one-stage software pipeline, interleaved per g:
                        # [S_g0(t2), ctx_g0(t2-1), S_g1(t2), ctx_g1(t2-1)].
                        # S_g(t2)'s PSUM bank frees when exp(t2-1, g)
                        # completes, so the g0 work is compute-ready
                        # mid-stage and exp(t2, g0) starts the moment the
                        # ACT queue frees - ACT never idles.
                        for t2 in range(nt2):
                            pts[t2] = [None, None]
                            for g in range(2):
                                pts[t2][g] = s_stage_g(t2, g)
                                if t2 == 0 and g == 0 and pending_tail:
                                    # previous group's last ctx + evacuation
                                    # runs under this group's first S tiles
                                    pending_tail.pop()()
                                if t2 >= 1:
                                    ctx_stage_g(t2 - 1, g)
                            # one filler matmul per stage: the ACT-bound
                            # cadence leaves the PE under the HAM activity
                            # threshold on the lighter (diagonal/short)
                            # stages, and a single re-throttle costs far
                            # more than 215ns/stage of filler
                            warm(1)
                            drain()

                        def group_tail(j5=j5, jh=jh, ctx_ps=ctx_ps,
                                       nt2=nt2, denS=denS, p=p,
                                       ctx_stage_g=ctx_stage_g):
                            for g in range(2):
                                ctx_stage_g(nt2 - 1, g)
                            # evacuate PSUM fast (frees the ctx banks for
                            # the next group); the reciprocal/divide runs
                            # later, overlapped under later compute
                            for g in range(2):
                                # [c, rc, m, 64jh + 16b + rrlo] <- ctx col
                                # (128b + 16m + rrlo)
                                nc.vector.tensor_copy(
                                    ctxP[p][64 * g:64 * (g + 1), j5 // 2, :,
                                            64 * jh:64 * (jh + 1)]
                                    .rearrange("c m (b rr) -> c m b rr",
                                               b=4),
                                    ctx_ps[g][0:64, :].rearrange(
                                        "c (b m rr) -> c m b rr",
                                        b=4, m=8))
                                nc.vector.tensor_copy(
                                    denS[32 * (2 * jh + g):
                                         32 * (2 * jh + g) + 1, :],
                                    ctx_ps[g][64:65, :])

                        pending_tail.append(group_tail)

                        if jh == 0:
                            rc = j5 // 2
                            recbox = []

                            def mkrec(denS=denS, recbox=recbox):
                                recS = mp.tile([128, 512], F32, tag="recS",
                                               name="recS")
                                # custom-DVE fast reciprocal (~18 bits,
                                # ~5x cheaper than InstReciprocal); one
                                # partition-parallel call covers all four
                                # staged denominator rows
                                nc.vector.reciprocal_approx_fast(
                                    recS[:], denS[:])
                                recbox.append(recS)

                            def divide(jh2, p=p, rc=rc, recbox=recbox):
                                recS = recbox[0]
                                # rec rows for both g at partitions 0/32,
                                # then ONE PE matmul with the gsel selector
                                # broadcasts them to partitions 0-63 /
                                # 64-127 in PSUM
                                reck2 = reck2s[jh2]
                                for g in range(2):
                                    k4 = 2 * jh2 + g
                                    nc.vector.tensor_copy(
                                        reck2[32 * g:32 * g + 1, :],
                                        recS[32 * k4:32 * k4 + 1, :])
                                rbc_ps = psop.tile([128, 512], F32,
                                                   tag="psO", name="rbcps")
                                nc.tensor.matmul(
                                    rbc_ps[:], gsel[:], reck2[0:33, :],
                                    start=True, stop=True)
                                for g in range(2):
                                    dst = ctxP[p][64 * g:64 * (g + 1),
                                                  rc, :,
                                                  64 * jh2:64 * (jh2 + 1)]
                                    nc.vector.tensor_tensor(
                                        dst.rearrange(
                                            "c m (b rr) -> c m b rr", b=4),
                                        dst.rearrange(
                                            "c m (b rr) -> c m b rr", b=4),
                                        rbc_ps[64 * g:64 * (g + 1), :]
                                        .rearrange(
                                            "c (b m rr) -> c m b rr",
                                            b=4, m=8),
                                        ALU.mult)

                            # stagger the deferred chain so consecutive
                            # psO-bank users never wait on each other's
                            # evacuation inside the PE FIFO; for p=1 only
                            # 6 drain ticks remain after the rc=1 pair, so
                            # compress the countdowns to fire in-loop
                            if p == 0:
                                cds = (3, 3, 5, 7, 9)
                            else:
                                cds = (2, 2, 3, 4, 6)
                            pending.append([cds[0], mkrec])
                            pending.append([cds[1], lambda d=divide: d(0)])
                            pending.append([cds[2], lambda d=divide: d(1)])
                            pending.append(
                                [cds[3],
                                 lambda p=p, rc=rc: out_proj(p, rc, 0)])
                            pending.append(
                                [cds[4],
                                 lambda p=p, rc=rc: out_proj(p, rc, 1)])
                # latency-bound tail: keep the PE clock warm between the
                # dependency-chained final divides / out-projections.
                # warms go BEFORE each item - emitted after, they would sit
                # behind the item's blocked matmuls in the PE FIFO and
                # never fill the dependency-wait gap.
                while pending_tail:
                    warm(4)
                    pending_tail.pop()()
                for item in pending:
                    warm(6)
                    item[1]()
                warm(4)

    nc.compile()
    return nc


def _get_nc():
    key = "nc"
    if key not in _cache:
        _cache[key] = _build()
    return _cache[key]


def pack_in_maps(x, Wq, bq, Wk, bk, Wv, bv, Wo, bo):
    BF = ml_dtypes.bfloat16
    x = np.asarray(x, np.float32)
    WqT = np.ascontiguousarray(np.asarray(Wq, np.float32).T.astype(BF))
    WkT = np.ascontiguousarray(np.asarray(Wk, np.float32).T.astype(BF))
    WvT = np.ascontiguousarray(np.asarray(Wv, np.float32).T.astype(BF))
    # woTre[64g + d, m, o] = Wo[o, 512g + 64m + d]
    WoTre = np.ascontiguousarray(
        np.asarray(Wo, np.float32).T.reshape(2, 8, 64, E).transpose(0, 2, 1, 3)
        .reshape(128, 8, E).astype(BF))
    bqT = np.ascontiguousarray(np.asarray(bq, np.float32).reshape(8, 128).T)
    bkT8 = np.ascontiguousarray((np.asarray(bk, np.float32) / 8.0).reshape(8, 128).T)
    bvrow = np.asarray(bv, np.float32).reshape(1, E)
    borow = np.asarray(bo, np.float32).reshape(1, E)
    # position-sorted 128-blocks, m-major in-block: index i = 16m + rr%16
    # has in-block position 8*(i%16) + i//16; one mask covers every
    # diagonal block
    ii = np.arange(128)[:, None]
    cc = np.arange(128)[None, :]
    pos_k = 8 * (ii % 16) + ii // 16
    pos_q = 8 * (cc % 16) + cc // 16
    masks = np.where(pos_k <= pos_q, 0.0, NEG).astype(BF)
    ident = np.eye(128).astype(BF)
    gsel = np.zeros((33, 128), np.float32)
    gsel[0, 0:64] = 1.0
    gsel[32, 64:128] = 1.0
    gsel = gsel.astype(BF)

    in_maps = []
    for c in range(8):
        xTs = np.empty((E, R), BF)
        for p in range(2):
            h = 2 * c + p
            b_, mp_ = divmod(h, 8)
            xTs[:, RP * p:RP * (p + 1)] = x[b_, RP * mp_:RP * (mp_ + 1), :].T.astype(BF)
        in_maps.append({
            "xT": np.ascontiguousarray(xTs), "wqT": WqT, "wkT": WkT,
            "wvT": WvT, "woTre": WoTre, "bqT": bqT, "bkT8": bkT8,
            "bvrow": bvrow, "borow": borow, "masks": masks, "ident": ident,
            "gsel": gsel,
        })
    return in_maps


def unpack_out(results):
    out = np.empty((2, 2048, E), np.float32)
    for c in range(8):
        o = results[c]["out"]
        for p in range(2):
            h = 2 * c + p
            b_, mp_ = divmod(h, 8)
            out[b_, RP * mp_:RP * (mp_ + 1), :] = o[RP * p:RP * (p + 1), :]
    return out


def kernel(x, Wq, bq, Wk, bk, Wv, bv, Wo, bo):
    in_maps = pack_in_maps(x, Wq, bq, Wk, bk, Wv, bv, Wo, bo)
    nc = _get_nc()
    res = run_bass_kernel_spmd(nc, in_maps, core_ids=list(range(8)))
    return unpack_out(res.results)
